# revision 12
# baseline (speedup 1.0000x reference)
"""Causal self-attention Trainium2 Bass kernel, data-parallel over 8 NeuronCores.

Problem (hardcoded): x [8, 2048, 1024] fp32; w_attn [1024, 3072]; b_attn [3072];
w_proj [1024, 1024]; b_proj [1024]. H=16 heads, D=64.

Sharding: batch (8) -> one sample per core. Each core runs the full
qkv-projection + causal attention + output projection for its [2048, 1024]
slice. Weights replicated.

Per-core algorithm (all layouts chosen so no transposes are needed apart from
one x -> xT PE transpose):
  - xT [C, T]   = x^T                       (PE transpose, 128x128 blocks)
  - qT/kT [C,T] = w_{q,k}^T @ x^T           (matmul: lhsT=w slice, rhs=xT)
  - v [T, C]    = x @ w_v                   (matmul: lhsT=xT slice, rhs=w_v)
  - S^T [tk,tq] = (k d,tk)^T... per head:     lhsT=kT_h [64,128], rhs=qT_h
                  two heads packed on the PE via row tiling (K=64 each).
  - P^T = exp(S^T * 1/sqrt(D)) with causal mask applied by a 0/1 multiply on
    the diagonal 128x128 blocks only; fully-masked regions never computed.
  - yT_h [64,tq] accumulated as lhsT=v_h [tk,64], rhs=P^T  (no P transpose!)
    two heads packed via output column tiling; softmax denominators ride as
    two extra M=1 ones-matmuls into a separate PSUM tile.
  - normalize: recip(denoms) -> PE broadcast via a constant selector matmul ->
    elementwise multiply.
  - out [T, C] = y @ w_proj (lhsT = yT chunks).

Matmuls use float32r (fp32 data, full-rate PE mode) via AP bitcast.
"""

import numpy as np
from contextlib import ExitStack

import concourse.bass as bass
import concourse.bacc as bacc
import concourse.tile as tile
from concourse import mybir
from concourse.bass_utils import run_bass_kernel_spmd

F32 = mybir.dt.float32
F32R = mybir.dt.float32r
P = 128


def _bank_slices(n0, qb_w):
    """Slices of [n0, qb_w) split at 512-element PSUM bank boundaries."""
    out = []
    s = n0
    while s < qb_w:
        s1 = min((s // 512 + 1) * 512, qb_w)
        out.append((s, s1))
        s = s1
    return out


def build_program(T=2048, C=1024, H=16, QB=1024, n_cores=8,
                  with_bias_attn=False, with_bias_proj=False):
    """Build + compile the per-core Bass program. Returns the Bacc module."""
    D = C // H
    assert D == 64 and H % 2 == 0
    assert C % P == 0 and T % P == 0
    QB = min(QB, T)
    assert T % QB == 0 and QB % 512 == 0 and QB <= 1024
    CIN = C // P          # contraction chunks of the input dim
    PAIRS = C // P        # head pairs (2 heads of 64 ch per 128-chunk)
    TKC = T // P          # key/time chunks
    NQB = T // QB
    TT = T // 512         # 512-wide t slices
    scale = 1.0 / float(np.sqrt(D))

    nc = bacc.Bacc("TRN2", target_bir_lowering=False, debug=False,
                   num_devices=n_cores)

    x_in = nc.dram_tensor("x", [T, C], F32, kind="ExternalInput")
    w_attn = nc.dram_tensor("w_attn", [C, 3 * C], F32R, kind="ExternalInput")
    w_proj = nc.dram_tensor("w_proj", [C, C], F32R, kind="ExternalInput")
    ident_in = nc.dram_tensor("ident", [P, P], F32, kind="ExternalInput")
    mask_in = nc.dram_tensor("mask", [P, P], F32R, kind="ExternalInput")
    sel_in = nc.dram_tensor("sel", [P, P], F32R, kind="ExternalInput")
    if with_bias_attn:
        b_attn = nc.dram_tensor("b_attn", [1, 3 * C], F32R, kind="ExternalInput")
    if with_bias_proj:
        b_proj = nc.dram_tensor("b_proj", [1, C], F32R, kind="ExternalInput")
    out_d = nc.dram_tensor("out", [T, C], F32, kind="ExternalOutput")

    # per-(q|k)-chunk DRAM staging tensors, so attention pair p only depends
    # on its own two tensors (fine-grained overlap with the qkv phase).
    qkT_d = [nc.dram_tensor(f"qkT{m}", [P, T], F32R) for m in range(2 * PAIRS)]

    with tile.TileContext(nc) as tc, ExitStack() as ctx:
        pool_c = ctx.enter_context(tc.tile_pool(name="const", bufs=1))
        ident_t = pool_c.tile([P, P], F32, tag="ident")
        mask_t = pool_c.tile([P, P], F32R, tag="mask")
        sel_t = pool_c.tile([P, P], F32R, tag="sel")
        nc.sync.dma_start(ident_t[:], ident_in[:])
        nc.sync.dma_start(mask_t[:], mask_in[:])
        nc.sync.dma_start(sel_t[:], sel_in[:])
        if with_bias_attn:
            ba_t = pool_c.tile([1, 3 * C], F32R, tag="ba")
            nc.sync.dma_start(ba_t[:], b_attn[:])
        if with_bias_proj:
            bp_t = pool_c.tile([1, C], F32R, tag="bp")
            nc.sync.dma_start(bp_t[:], b_proj[:])
        if with_bias_attn or with_bias_proj:
            ones_row = pool_c.tile([1, 512], F32R, tag="ones_row")
            nc.gpsimd.memset(ones_row[:], 1.0)

        # v stays resident in SBUF: v_t[i][:, h, 0:64] = v[128i:128i+128, 64h:64h+64],
        # v_t[i][:, h, 64] = 1.0 (rides the PV matmul to produce softmax denoms)
        pool_v = ctx.enter_context(tc.tile_pool(name="vres", bufs=1))
        v_t = [pool_v.tile([P, H, D + 1], F32R, tag=f"v{i}", name=f"v{i}")
               for i in range(TKC)]
        ones_H = pool_c.tile([P, H, 1], F32, tag="ones_H")
        nc.gpsimd.memset(ones_H[:], 1.0)
        # attention-phase psum pool opened FIRST so it does not overlap the
        # qkv-phase psum pools (lets attention S/exp overlap the qkv tail).
        pool_ps_s = ctx.enter_context(
            tc.tile_pool(name="ps_s", bufs=2, space="PSUM"))

        # ---------------- phase 0: x -> xT ----------------
        with ExitStack() as phx:
            pool_xT = phx.enter_context(tc.tile_pool(name="xT", bufs=1))
            xT = [pool_xT.tile([P, T], F32R, tag=f"xT{j}", name=f"xT{j}")
                  for j in range(CIN)]
            with ExitStack() as ph0:
                pool_xl = ph0.enter_context(tc.tile_pool(name="xload", bufs=3))
                pool_ps0 = ph0.enter_context(
                    tc.tile_pool(name="ps_tr", bufs=2, space="PSUM"))
                for i in range(T // P):
                    xl = pool_xl.tile([P, C], F32, tag="xl", name="xl")
                    nc.sync.dma_start(xl[:], x_in[i * P:(i + 1) * P, :])
                    for j in range(CIN):
                        ps = pool_ps0.tile([P, P], F32, tag="tr", name="ps_tr")
                        nc.tensor.transpose(ps[:], xl[:, j * P:(j + 1) * P],
                                            ident_t[:])
                        nc.vector.tensor_copy(xT[j][:, i * P:(i + 1) * P], ps[:])

            # ---------------- phase 1: qkv projection ----------------
            with ExitStack() as ph1:
                pool_wqk = ph1.enter_context(tc.tile_pool(name="wqk", bufs=2))
                pool_wv = ph1.enter_context(tc.tile_pool(name="wv", bufs=1))
                pool_st = ph1.enter_context(tc.tile_pool(name="qkst", bufs=4))
                pool_ps_qk = ph1.enter_context(
                    tc.tile_pool(name="ps_qk", bufs=2, space="PSUM"))
                pool_ps_v = ph1.enter_context(
                    tc.tile_pool(name="ps_v", bufs=2, space="PSUM"))

                wv_t = [pool_wv.tile([P, C], F32R, tag=f"wv{j}", name=f"wv{j}")
                        for j in range(CIN)]
                for j in range(CIN):
                    nc.sync.dma_start(
                        wv_t[j][:], w_attn[j * P:(j + 1) * P, 2 * C:3 * C])

                def emit_qk_chunk(m):
                    # output channels: q chunk m (m < PAIRS) / k chunk m-PAIRS
                    col0 = m * P if m < PAIRS else C + (m - PAIRS) * P
                    wm = pool_wqk.tile([P, CIN, P], F32R, tag="wqk", name="wm")
                    nc.sync.dma_start(
                        wm[:],
                        w_attn[:, col0:col0 + P].rearrange(
                            "(j p) n -> p j n", p=P))
                    for tt in range(TT):
                        ps = pool_ps_qk.tile([P, 512], F32, tag="qk", name="ps_qk")
                        for j in range(CIN):
                            nc.tensor.matmul(
                                ps[:], wm[:, j, :],
                                xT[j][:, tt * 512:(tt + 1) * 512],
                                start=(j == 0),
                                stop=(j == CIN - 1 and not with_bias_attn))
                        if with_bias_attn:
                            nc.tensor.matmul(
                                ps[:], ba_t[0:1, col0:col0 + P],
                                ones_row[0:1, :],
                                start=False, stop=True)
                        st = pool_st.tile([P, 512], F32R, tag="st", name="st")
                        nc.vector.tensor_copy(st[:], ps[:])
                        nc.sync.dma_start(
                            qkT_d[m][:, tt * 512:(tt + 1) * 512], st[:])

                def emit_v_chunk(i):
                    for g in range(0, C, 512):
                        gw = min(512, C - g)
                        ps = pool_ps_v.tile([P, 512], F32, tag="v", name="ps_v")
                        for j in range(CIN):
                            nc.tensor.matmul(
                                ps[:, 0:gw],
                                xT[j][:, i * P:(i + 1) * P],
                                wv_t[j][:, g:g + gw],
                                start=(j == 0),
                                stop=(j == CIN - 1 and not with_bias_attn))
                        if with_bias_attn:
                            nc.tensor.matmul(
                                ps[:, 0:gw], ones_row[0:1, 0:P],
                                ba_t[0:1, 2 * C + g:2 * C + g + gw],
                                start=False, stop=True)
                        nc.vector.tensor_copy(
                            v_t[i][:, g // D:(g + gw) // D, 0:D],
                            ps[:, 0:gw].rearrange("p (h d) -> p h d", d=D))

                for i in range(TKC):
                    nc.vector.tensor_copy(v_t[i][:, :, D:D + 1], ones_H[:])

                # interleave: per pair its q and k chunks, then a slab of v
                vi = 0
                for pr in range(PAIRS):
                    emit_qk_chunk(pr)
                    emit_qk_chunk(PAIRS + pr)
                    vn = (TKC * (pr + 1) + PAIRS - 1) // PAIRS
                    while vi < vn:
                        emit_v_chunk(vi)
                        vi += 1

        # ---------------- phase 2: attention ----------------
        # normalized yT, resident until the output projection; opened after
        # the xT pool is released so the two never coexist in SBUF.
        pool_y = ctx.enter_context(tc.tile_pool(name="yres", bufs=1))
        yT_sb = [pool_y.tile([P, T], F32R, tag=f"y{j}", name=f"y{j}")
                 for j in range(PAIRS)]
        ph2 = ctx.enter_context(ExitStack())
        pool_qkp = ph2.enter_context(tc.tile_pool(name="qkpair", bufs=2))
        pool_exp = ph2.enter_context(tc.tile_pool(name="expS", bufs=4))
        pool_yst = ph2.enter_context(tc.tile_pool(name="yst", bufs=2))
        pool_rc = ph2.enter_context(tc.tile_pool(name="recip", bufs=1))
        pool_ps_y = ph2.enter_context(
            tc.tile_pool(name="ps_y", bufs=1, space="PSUM"))

        recip_e = pool_rc.tile([P, QB], F32R, tag="recip_e")
        recip_o = pool_rc.tile([P, QB], F32R, tag="recip_o")
        rz = pool_yst.tile([P, QB], F32, tag="yst", name="rz")
        nc.gpsimd.memset(rz[:], 0.0)
        nc.vector.tensor_copy(recip_e[:], rz[:])
        nc.vector.tensor_copy(recip_o[:], rz[:])

        for pr in range(PAIRS):
            qT_p = pool_qkp.tile([P, T], F32R, tag="qTp", name="qT_p")
            kT_p = pool_qkp.tile([P, T], F32R, tag="kTp", name="kT_p")
            nc.sync.dma_start(qT_p[:], qkT_d[pr][:])
            nc.sync.dma_start(kT_p[:], qkT_d[PAIRS + pr][:])
            for qb in range(NQB):
                q0 = qb * QB
                c_hi = (q0 + QB) // P - 1
                yT_e_ps = pool_ps_y.tile([P, QB], F32, tag="y_e", name="yT_e_ps")
                yT_o_ps = pool_ps_y.tile([P, QB], F32, tag="y_o", name="yT_o_ps")
                for c in range(c_hi + 1):
                    n0 = max(0, c * P - q0)
                    sl = _bank_slices(n0, QB)
                    sT_e = pool_ps_s.tile([P, QB], F32, tag="sT", name="sT_e")
                    sT_o = pool_ps_s.tile([P, QB], F32, tag="sT", name="sT_o")
                    for (s0, s1) in sl:
                        nc.tensor.matmul(
                            sT_e[:, s0:s1],
                            kT_p[0:D, c * P:(c + 1) * P],
                            qT_p[0:D, q0 + s0:q0 + s1],
                            start=True, stop=True, tile_position=(0, 0))
                        nc.tensor.matmul(
                            sT_o[:, s0:s1],
                            kT_p[D:2 * D, c * P:(c + 1) * P],
                            qT_p[D:2 * D, q0 + s0:q0 + s1],
                            start=True, stop=True, tile_position=(D, 0))
                    ex_e = pool_exp.tile([P, QB], F32R, tag="ex", name="ex_e")
                    ex_o = pool_exp.tile([P, QB], F32R, tag="ex", name="ex_o")
                    nc.scalar.activation(ex_e[:, n0:QB], sT_e[:, n0:QB],
                                         mybir.ActivationFunctionType.Exp,
                                         scale=scale)
                    nc.scalar.activation(ex_o[:, n0:QB], sT_o[:, n0:QB],
                                         mybir.ActivationFunctionType.Exp,
                                         scale=scale)
                    if c * P >= q0:  # diagonal block: causal 0/1 mask
                        nc.vector.tensor_mul(ex_e[:, n0:n0 + P],
                                             ex_e[:, n0:n0 + P], mask_t[:])
                        nc.vector.tensor_mul(ex_o[:, n0:n0 + P],
                                             ex_o[:, n0:n0 + P], mask_t[:])
                    for (s0, s1) in sl:
                        c_last = min(c_hi, (q0 + s1) // P - 1)
                        st_ = (c == 0)
                        sp_ = (c == c_last)
                        nc.tensor.matmul(
                            yT_e_ps[0:D + 1, s0:s1],
                            v_t[c][:, 2 * pr, :],
                            ex_e[:, s0:s1],
                            start=st_, stop=sp_, skip_group_check=True)
                        nc.tensor.matmul(
                            yT_o_ps[0:D + 1, s0:s1],
                            v_t[c][:, 2 * pr + 1, :],
                            ex_o[:, s0:s1],
                            start=st_, stop=sp_, skip_group_check=True)
                # normalize: yT_sb[pr] rows 0:64 (even head) / 64:128 (odd)
                # = yT_ps rows 0:64 divided by the denom row 64.
                with nc.allow_low_precision(reason="f32r matmul operands"):
                    nc.vector.reciprocal(recip_e[D:D + 1, :], yT_e_ps[D:D + 1, :])
                    nc.vector.reciprocal(recip_o[D:D + 1, :], yT_o_ps[D:D + 1, :])
                bc_e = pool_ps_y.tile([P, QB], F32, tag="y_e", name="bc_e")
                bc_o = pool_ps_y.tile([P, QB], F32, tag="y_o", name="bc_o")
                for (s0, s1) in _bank_slices(0, QB):
                    nc.tensor.matmul(bc_e[0:D, s0:s1], sel_t[:, 0:D],
                                     recip_e[:, s0:s1], start=True, stop=True)
                    nc.tensor.matmul(bc_o[0:D, s0:s1], sel_t[:, 0:D],
                                     recip_o[:, s0:s1], start=True, stop=True)
                yst_e = pool_yst.tile([P, QB], F32, tag="yst", name="yst_e")
                yst_o = pool_yst.tile([P, QB], F32, tag="yst", name="yst_o")
                nc.scalar.copy(yst_e[0:D, :], yT_e_ps[0:D, :])
                nc.scalar.copy(yst_o[0:D, :], yT_o_ps[0:D, :])
                nc.vector.tensor_mul(yT_sb[pr][0:D, q0:q0 + QB],
                                     yst_e[0:D, :], bc_e[0:D, :])
                tmp_o = pool_yst.tile([P, QB], F32R, tag="tmp_o", name="tmp_o")
                nc.vector.tensor_mul(tmp_o[0:D, :], yst_o[0:D, :], bc_o[0:D, :])
                nc.sync.dma_start(yT_sb[pr][D:P, q0:q0 + QB], tmp_o[0:D, :])

        ph2.close()

        # ---------------- phase 3: output projection ----------------
        with ExitStack() as ph3:
            pool_wp = ph3.enter_context(tc.tile_pool(name="wp", bufs=1))
            pool_ost = ph3.enter_context(tc.tile_pool(name="ost", bufs=3))
            pool_ps_o = ph3.enter_context(
                tc.tile_pool(name="ps_o", bufs=4, space="PSUM"))
            wp_t = [pool_wp.tile([P, C], F32R, tag=f"wp{j}", name=f"wp{j}")
                    for j in range(CIN)]
            for j in range(CIN):
                nc.sync.dma_start(wp_t[j][:], w_proj[j * P:(j + 1) * P, :])
            for i in range(T // P):
                ost = pool_ost.tile([P, C], F32, tag="ost", name="ost")
                for g in range(0, C, 512):
                    gw = min(512, C - g)
                    ps = pool_ps_o.tile([P, 512], F32, tag="o", name="ps_o")
                    for j in range(CIN):
                        nc.tensor.matmul(
                            ps[:, 0:gw],
                            yT_sb[j][:, i * P:(i + 1) * P],
                            wp_t[j][:, g:g + gw],
                            start=(j == 0),
                            stop=(j == CIN - 1 and not with_bias_proj))
                    if with_bias_proj:
                        nc.tensor.matmul(
                            ps[:, 0:gw], ones_row[0:1, 0:P],
                            bp_t[0:1, g:g + gw],
                            start=False, stop=True)
                    nc.vector.tensor_copy(ost[:, g:g + gw], ps[:, 0:gw])
                nc.sync.dma_start(out_d[i * P:(i + 1) * P, :], ost[:])

    nc.compile()
    return nc


def make_const_inputs():
    ident = np.eye(P, dtype=np.float32)
    # S^T diagonal block mask: valid iff tq_local >= tk_local
    mask = np.triu(np.ones((P, P), dtype=np.float32))
    # broadcast selector: denom row 64 -> all 64 output rows
    sel = np.zeros((P, P), dtype=np.float32)
    sel[64, 0:64] = 1.0
    return ident, mask, sel


_CACHE = {}


def _get_program(T, C, H, with_bias_attn, with_bias_proj, n_cores):
    key = (T, C, H, with_bias_attn, with_bias_proj, n_cores)
    if key not in _CACHE:
        _CACHE[key] = build_program(T=T, C=C, H=H, n_cores=n_cores,
                                    with_bias_attn=with_bias_attn,
                                    with_bias_proj=with_bias_proj)
    return _CACHE[key]


def kernel(x, w_attn, b_attn, w_proj, b_proj):
    x = np.ascontiguousarray(np.asarray(x, dtype=np.float32))
    w_attn = np.ascontiguousarray(np.asarray(w_attn, dtype=np.float32))
    w_proj = np.ascontiguousarray(np.asarray(w_proj, dtype=np.float32))
    b_attn = np.asarray(b_attn, dtype=np.float32)
    b_proj = np.asarray(b_proj, dtype=np.float32)
    B, T, C = x.shape
    H = 16
    n_cores = 8
    assert B == n_cores

    wba = bool(np.any(b_attn != 0))
    wbp = bool(np.any(b_proj != 0))
    nc = _get_program(T, C, H, wba, wbp, n_cores)

    ident, mask, sel = make_const_inputs()
    in_maps = []
    for i in range(n_cores):
        m = {"x": x[i], "w_attn": w_attn, "w_proj": w_proj,
             "ident": ident, "mask": mask, "sel": sel}
        if wba:
            m["b_attn"] = b_attn.reshape(1, -1)
        if wbp:
            m["b_proj"] = b_proj.reshape(1, -1)
        in_maps.append(m)

    res = run_bass_kernel_spmd(nc, in_maps, list(range(n_cores)))
    return np.stack([res.results[i]["out"] for i in range(n_cores)], axis=0)


# revision 21
# speedup vs baseline: 36.7057x; 36.7057x over previous
"""Causal self-attention Trainium2 Bass kernel, data-parallel over 8 NeuronCores.

Problem (hardcoded): x [8, 2048, 1024] fp32; w_attn [1024, 3072]; b_attn [3072];
w_proj [1024, 1024]; b_proj [1024]. H=16 heads, D=64.

Sharding: batch (8) -> one sample per core. Each core runs the full
qkv-projection + causal attention + output projection for its [2048, 1024]
slice. Weights replicated.

Per-core algorithm (layouts chosen so the only transpose is x -> xT, done
once on the PE):
  - xT [C, T]   = x^T                       (PE transpose, 128x128 blocks)
  - qT/kT [C,T] = w_{q,k}^T @ x^T           (matmul: lhsT=w slice, rhs=xT)
  - v [T, C]    = x @ w_v                   (matmul: lhsT=xT slice, rhs=w_v)
  - S^T [tk,tq] per head: lhsT=kT_h [64, tk chunk], rhs=qT_h [64, tq]
    (head pairs packed on the PE via row tiling: K=64 at partitions 0/64)
  - P^T = exp(S^T / sqrt(D)) on ScalarE (scale folded into the activation);
    causal mask = 0/1 multiply on diagonal 128-blocks only; fully-masked
    regions are never computed (matmul/exp restricted to the causal range).
  - yT_h accumulated over tk chunks as lhsT=v_aug_h [tk,65] (64 v channels
    + a ones column whose output row is the softmax denominator), rhs=P^T.
    No P transpose anywhere.
  - normalize: DVE reciprocal of the denominator row, PE broadcast via a
    constant selector matmul, elementwise multiply; odd-head halves moved
    to partitions 64..127 with an SBUF->SBUF DMA (fp32r matmuls cannot
    col-tile to a nonzero dst partition).
  - out [T, C] = y @ w_proj (lhsT = yT chunks).

All matmul operands are float32r (fp32 data, full-rate PE mode; every
producer writes f32r so the BIR verifier sees rounded inputs). Measured on
HW: max rel err 2.6e-4 vs the fp32 jax reference.
"""

import numpy as np
from contextlib import ExitStack

import concourse.bacc as bacc
import concourse.tile as tile
from concourse import mybir
from concourse.bass_utils import run_bass_kernel_spmd

F32 = mybir.dt.float32
F32R = mybir.dt.float32r
P = 128


def _bank_slices(n0, qb_w):
    """Slices of [n0, qb_w) split at 512-element PSUM bank boundaries."""
    out = []
    s = n0
    while s < qb_w:
        s1 = min((s // 512 + 1) * 512, qb_w)
        out.append((s, s1))
        s = s1
    return out


def build_program(T=2048, C=1024, H=16, QB=1024, n_cores=8,
                  with_bias_attn=False, with_bias_proj=False,
                  phases=("qkv", "attn", "proj")):
    """Build + compile the per-core Bass program. Returns the Bacc module."""
    D = C // H
    assert D == 64 and H % 2 == 0
    assert C % P == 0 and T % P == 0
    QB = min(QB, T)
    assert T % QB == 0 and QB % 512 == 0 and QB <= 1024
    CIN = C // P          # contraction chunks of the input dim
    PAIRS = C // P        # head pairs (2 heads of 64 ch per 128-chunk)
    TKC = T // P          # key/time chunks
    NQB = T // QB
    TT = T // 512         # 512-wide t slices
    scale = 1.0 / float(np.sqrt(D))

    nc = bacc.Bacc("TRN2", target_bir_lowering=False, debug=False,
                   num_devices=n_cores)

    x_in = nc.dram_tensor("x", [T, C], F32, kind="ExternalInput")
    w_attn = nc.dram_tensor("w_attn", [C, 3 * C], F32R, kind="ExternalInput")
    w_proj = nc.dram_tensor("w_proj", [C, C], F32R, kind="ExternalInput")
    ident_in = nc.dram_tensor("ident", [P, P], F32, kind="ExternalInput")
    mask_in = nc.dram_tensor("mask", [P, P], F32R, kind="ExternalInput")
    sel_in = nc.dram_tensor("sel", [P, P], F32R, kind="ExternalInput")
    if with_bias_attn:
        b_attn = nc.dram_tensor("b_attn", [1, 3 * C], F32R, kind="ExternalInput")
    if with_bias_proj:
        b_proj = nc.dram_tensor("b_proj", [1, C], F32R, kind="ExternalInput")
    out_d = nc.dram_tensor("out", [T, C], F32, kind="ExternalOutput")

    # per-(q|k)-chunk DRAM staging tensors, so attention pair p only depends
    # on its own two tensors (fine-grained overlap with the qkv phase).
    qkT_d = [nc.dram_tensor(f"qkT{m}", [P, T], F32R) for m in range(2 * PAIRS)]

    with tile.TileContext(nc) as tc, ExitStack() as ctx:
        pool_c = ctx.enter_context(tc.tile_pool(name="const", bufs=1))
        ident_t = pool_c.tile([P, P], F32, tag="ident")
        mask_t = pool_c.tile([P, P], F32R, tag="mask")
        sel_t = pool_c.tile([P, P], F32R, tag="sel")
        nc.sync.dma_start(ident_t[:], ident_in[:])
        nc.sync.dma_start(mask_t[:], mask_in[:])
        nc.sync.dma_start(sel_t[:], sel_in[:])
        if with_bias_attn:
            ba_t = pool_c.tile([1, 3 * C], F32R, tag="ba")
            nc.sync.dma_start(ba_t[:], b_attn[:])
        if with_bias_proj:
            bp_t = pool_c.tile([1, C], F32R, tag="bp")
            nc.sync.dma_start(bp_t[:], b_proj[:])
        if with_bias_attn or with_bias_proj:
            ones_row = pool_c.tile([1, 512], F32R, tag="ones_row")
            nc.gpsimd.memset(ones_row[:], 1.0)

        # v stays resident in SBUF: v_t[i][:, h, 0:64] = v[128i:128i+128, 64h:64h+64],
        # v_t[i][:, h, 64] = 1.0 (rides the PV matmul to produce softmax denoms)
        pool_v = ctx.enter_context(tc.tile_pool(name="vres", bufs=1))
        v_t = [pool_v.tile([P, H, D + 1], F32R, tag=f"v{i}", name=f"v{i}")
               for i in range(TKC)]
        ones_H = pool_c.tile([P, H, 1], F32, tag="ones_H")
        nc.gpsimd.memset(ones_H[:], 1.0)
        # attention-phase psum pool opened FIRST so it does not overlap the
        # qkv-phase psum pools (lets attention S/exp overlap the qkv tail).
        pool_ps_s = ctx.enter_context(
            tc.tile_pool(name="ps_s", bufs=2, space="PSUM"))
        # attention SBUF pools pre-opened for the same reason: placed below
        # the phase-0/1 pools so their allocation does not wait on phase-1
        # pool releases.
        pool_qkp = ctx.enter_context(tc.tile_pool(name="qkpair", bufs=2))
        pool_exp = ctx.enter_context(tc.tile_pool(name="expS", bufs=4))

        # ---------------- phase 0: x -> xT ----------------
        with ExitStack() as phx:
            pool_xT = phx.enter_context(tc.tile_pool(name="xT", bufs=1))
            xT = [pool_xT.tile([P, T], F32R, tag=f"xT{j}", name=f"xT{j}")
                  for j in range(CIN)]
            with ExitStack() as ph0:
                pool_xl = ph0.enter_context(tc.tile_pool(name="xload", bufs=2))
                pool_ps0 = ph0.enter_context(
                    tc.tile_pool(name="ps_tr", bufs=2, space="PSUM"))
                for i in range(T // P):
                    xl = pool_xl.tile([P, C], F32, tag="xl", name="xl")
                    nc.sync.dma_start(xl[:], x_in[i * P:(i + 1) * P, :])
                    for j in range(CIN):
                        ps = pool_ps0.tile([P, P], F32, tag="tr", name="ps_tr")
                        nc.tensor.transpose(ps[:], xl[:, j * P:(j + 1) * P],
                                            ident_t[:])
                        nc.vector.tensor_copy(xT[j][:, i * P:(i + 1) * P], ps[:])

            # ---------------- phase 1: qkv projection ----------------
            with ExitStack() as ph1:
                pool_wqk = ph1.enter_context(tc.tile_pool(name="wqk", bufs=2))
                pool_wv = ph1.enter_context(tc.tile_pool(name="wv", bufs=1))
                pool_st = ph1.enter_context(tc.tile_pool(name="qkst", bufs=2))
                pool_ps_qk = ph1.enter_context(
                    tc.tile_pool(name="ps_qk", bufs=2, space="PSUM"))
                pool_ps_v = ph1.enter_context(
                    tc.tile_pool(name="ps_v", bufs=2, space="PSUM"))



                def emit_qk_chunk(m):
                    # output channels: q chunk m (m < PAIRS) / k chunk m-PAIRS
                    col0 = m * P if m < PAIRS else C + (m - PAIRS) * P
                    wm = pool_wqk.tile([P, CIN, P], F32R, tag="wqk", name="wm")
                    nc.sync.dma_start(
                        wm[:],
                        w_attn[:, col0:col0 + P].rearrange(
                            "(j p) n -> p j n", p=P))
                    for tt in range(TT):
                        ps = pool_ps_qk.tile([P, 512], F32, tag="qk", name="ps_qk")
                        for j in range(CIN):
                            nc.tensor.matmul(
                                ps[:], wm[:, j, :],
                                xT[j][:, tt * 512:(tt + 1) * 512],
                                start=(j == 0),
                                stop=(j == CIN - 1 and not with_bias_attn))
                        if with_bias_attn:
                            nc.tensor.matmul(
                                ps[:], ba_t[0:1, col0:col0 + P],
                                ones_row[0:1, :],
                                start=False, stop=True)
                        st = pool_st.tile([P, 512], F32R, tag="st", name="st")
                        nc.vector.tensor_copy(st[:], ps[:])
                        nc.sync.dma_start(
                            qkT_d[m][:, tt * 512:(tt + 1) * 512], st[:])

                def emit_v_group(g):
                    # one 512-wide slab of output channels for ALL t-chunks;
                    # wv tiles for the slab are streamed (16KB/p resident)
                    gw = min(512, C - g)
                    wv_t = []
                    for j in range(CIN):
                        wv = pool_wv.tile([P, 512], F32R, tag=f"wv{j}",
                                          name=f"wv{j}")
                        nc.sync.dma_start(
                            wv[:, 0:gw],
                            w_attn[j * P:(j + 1) * P, 2 * C + g:2 * C + g + gw])
                        wv_t.append(wv)
                    for i in range(TKC):
                        ps = pool_ps_v.tile([P, 512], F32, tag="v", name="ps_v")
                        for j in range(CIN):
                            nc.tensor.matmul(
                                ps[:, 0:gw],
                                xT[j][:, i * P:(i + 1) * P],
                                wv_t[j][:, 0:gw],
                                start=(j == 0),
                                stop=(j == CIN - 1 and not with_bias_attn))
                        if with_bias_attn:
                            nc.tensor.matmul(
                                ps[:, 0:gw], ones_row[0:1, 0:P],
                                ba_t[0:1, 2 * C + g:2 * C + g + gw],
                                start=False, stop=True)
                        nc.vector.tensor_copy(
                            v_t[i][:, g // D:(g + gw) // D, 0:D],
                            ps[:, 0:gw].rearrange("p (h d) -> p h d", d=D))
                        nc.vector.tensor_copy(
                            v_t[i][:, g // D:(g + gw) // D, D:D + 1],
                            ones_H[:, g // D:(g + gw) // D, :])

                # interleave: qk chunks for early pairs first, v slabs next,
                # remaining qk chunks after (attention pair p needs qkT pair p
                # and the v slab covering its channels)
                groups = list(range(0, C, 512))
                emit_qk_chunk(0)
                emit_qk_chunk(PAIRS)
                emit_v_group(groups[0])
                pr_done = 1
                for g in groups[1:]:
                    emit_qk_chunk(pr_done)
                    emit_qk_chunk(PAIRS + pr_done)
                    pr_done += 1
                    emit_v_group(g)
                for pr in range(pr_done, PAIRS):
                    emit_qk_chunk(pr)
                    emit_qk_chunk(PAIRS + pr)

        # ---------------- phase 2: attention ----------------
        # normalized yT, resident until the output projection; opened after
        # the xT pool is released so the two never coexist in SBUF.
        pool_y = ctx.enter_context(tc.tile_pool(name="yres", bufs=1))
        yT_sb = [pool_y.tile([P, T], F32R, tag=f"y{j}", name=f"y{j}")
                 for j in range(PAIRS)]
        ph2 = ctx.enter_context(ExitStack())
        pool_yst = ph2.enter_context(tc.tile_pool(name="yst", bufs=2))
        pool_rc = ph2.enter_context(tc.tile_pool(name="recip", bufs=1))
        pool_ps_y = ph2.enter_context(
            tc.tile_pool(name="ps_y", bufs=1, space="PSUM"))

        recip_e = pool_rc.tile([P, QB], F32R, tag="recip_e")
        recip_o = pool_rc.tile([P, QB], F32R, tag="recip_o")
        rz = pool_yst.tile([P, QB], F32, tag="yst", name="rz")
        nc.gpsimd.memset(rz[:], 0.0)
        nc.vector.tensor_copy(recip_e[:], rz[:])
        nc.vector.tensor_copy(recip_o[:], rz[:])

        for pr in (range(PAIRS) if "attn" in phases else []):
            qT_p = pool_qkp.tile([P, T], F32R, tag="qTp", name="qT_p")
            kT_p = pool_qkp.tile([P, T], F32R, tag="kTp", name="kT_p")
            nc.sync.dma_start(qT_p[:], qkT_d[pr][:])
            nc.sync.dma_start(kT_p[:], qkT_d[PAIRS + pr][:])
            for qb in range(NQB):
                q0 = qb * QB
                c_hi = (q0 + QB) // P - 1
                yT_e_ps = pool_ps_y.tile([P, QB], F32, tag="y_e", name="yT_e_ps")
                yT_o_ps = pool_ps_y.tile([P, QB], F32, tag="y_o", name="yT_o_ps")
                for c in range(c_hi + 1):
                    n0 = max(0, c * P - q0)
                    sl = _bank_slices(n0, QB)
                    sT_e = pool_ps_s.tile([P, QB], F32, tag="sT", name="sT_e")
                    sT_o = pool_ps_s.tile([P, QB], F32, tag="sT", name="sT_o")
                    for (s0, s1) in sl:
                        nc.tensor.matmul(
                            sT_e[:, s0:s1],
                            kT_p[0:D, c * P:(c + 1) * P],
                            qT_p[0:D, q0 + s0:q0 + s1],
                            start=True, stop=True, tile_position=(0, 0))
                        nc.tensor.matmul(
                            sT_o[:, s0:s1],
                            kT_p[D:2 * D, c * P:(c + 1) * P],
                            qT_p[D:2 * D, q0 + s0:q0 + s1],
                            start=True, stop=True, tile_position=(D, 0))
                    ex_e = pool_exp.tile([P, QB], F32R, tag="ex", name="ex_e")
                    ex_o = pool_exp.tile([P, QB], F32R, tag="ex", name="ex_o")
                    nc.scalar.activation(ex_e[:, n0:QB], sT_e[:, n0:QB],
                                         mybir.ActivationFunctionType.Exp,
                                         scale=scale)
                    nc.scalar.activation(ex_o[:, n0:QB], sT_o[:, n0:QB],
                                         mybir.ActivationFunctionType.Exp,
                                         scale=scale)
                    if c * P >= q0:  # diagonal block: causal 0/1 mask
                        nc.vector.tensor_mul(ex_e[:, n0:n0 + P],
                                             ex_e[:, n0:n0 + P], mask_t[:])
                        nc.vector.tensor_mul(ex_o[:, n0:n0 + P],
                                             ex_o[:, n0:n0 + P], mask_t[:])
                    for (s0, s1) in sl:
                        c_last = min(c_hi, (q0 + s1) // P - 1)
                        st_ = (c == 0)
                        sp_ = (c == c_last)
                        nc.tensor.matmul(
                            yT_e_ps[0:D + 1, s0:s1],
                            v_t[c][:, 2 * pr, :],
                            ex_e[:, s0:s1],
                            start=st_, stop=sp_, skip_group_check=True)
                        nc.tensor.matmul(
                            yT_o_ps[0:D + 1, s0:s1],
                            v_t[c][:, 2 * pr + 1, :],
                            ex_o[:, s0:s1],
                            start=st_, stop=sp_, skip_group_check=True)
                # normalize: yT_sb[pr] rows 0:64 (even head) / 64:128 (odd)
                # = yT_ps rows 0:64 divided by the denom row 64.
                with nc.allow_low_precision(reason="f32r matmul operands"):
                    nc.vector.reciprocal(recip_e[D:D + 1, :], yT_e_ps[D:D + 1, :])
                    nc.vector.reciprocal(recip_o[D:D + 1, :], yT_o_ps[D:D + 1, :])
                bc_e = pool_ps_y.tile([P, QB], F32, tag="y_e", name="bc_e")
                bc_o = pool_ps_y.tile([P, QB], F32, tag="y_o", name="bc_o")
                for (s0, s1) in _bank_slices(0, QB):
                    nc.tensor.matmul(bc_e[0:D, s0:s1], sel_t[:, 0:D],
                                     recip_e[:, s0:s1], start=True, stop=True)
                    nc.tensor.matmul(bc_o[0:D, s0:s1], sel_t[:, 0:D],
                                     recip_o[:, s0:s1], start=True, stop=True)
                yst_e = pool_yst.tile([P, QB], F32, tag="yst", name="yst_e")
                yst_o = pool_yst.tile([P, QB], F32, tag="yst", name="yst_o")
                nc.scalar.copy(yst_e[0:D, :], yT_e_ps[0:D, :])
                nc.scalar.copy(yst_o[0:D, :], yT_o_ps[0:D, :])
                nc.vector.tensor_mul(yT_sb[pr][0:D, q0:q0 + QB],
                                     yst_e[0:D, :], bc_e[0:D, :])
                tmp_o = pool_yst.tile([P, QB], F32R, tag="tmp_o", name="tmp_o")
                nc.vector.tensor_mul(tmp_o[0:D, :], yst_o[0:D, :], bc_o[0:D, :])
                nc.sync.dma_start(yT_sb[pr][D:P, q0:q0 + QB], tmp_o[0:D, :])

        ph2.close()

        # ---------------- phase 3: output projection ----------------
        with ExitStack() as ph3:
            pool_wp = ph3.enter_context(tc.tile_pool(name="wp", bufs=1))
            pool_ost = ph3.enter_context(tc.tile_pool(name="ost", bufs=4))
            pool_ps_o = ph3.enter_context(
                tc.tile_pool(name="ps_o", bufs=4, space="PSUM"))
            for g in (range(0, C, 512) if "proj" in phases else []):
                gw = min(512, C - g)
                wp_t = []
                for j in range(CIN):
                    wp = pool_wp.tile([P, 512], F32R, tag=f"wp{j}",
                                      name=f"wp{j}")
                    nc.sync.dma_start(wp[:, 0:gw],
                                      w_proj[j * P:(j + 1) * P, g:g + gw])
                    wp_t.append(wp)
                for i in range(T // P):
                    ps = pool_ps_o.tile([P, 512], F32, tag="o", name="ps_o")
                    for j in range(CIN):
                        nc.tensor.matmul(
                            ps[:, 0:gw],
                            yT_sb[j][:, i * P:(i + 1) * P],
                            wp_t[j][:, 0:gw],
                            start=(j == 0),
                            stop=(j == CIN - 1 and not with_bias_proj))
                    if with_bias_proj:
                        nc.tensor.matmul(
                            ps[:, 0:gw], ones_row[0:1, 0:P],
                            bp_t[0:1, g:g + gw],
                            start=False, stop=True)
                    ost = pool_ost.tile([P, 512], F32, tag="ost", name="ost")
                    nc.vector.tensor_copy(ost[:, 0:gw], ps[:, 0:gw])
                    nc.sync.dma_start(out_d[i * P:(i + 1) * P, g:g + gw],
                                      ost[:, 0:gw])

        if "proj" not in phases:
            with tc.tile_pool(name="fill", bufs=2) as pf:
                for i in range(T // P):
                    t0_ = pf.tile([P, C], F32, tag="f", name="f")
                    nc.sync.dma_start(t0_[:], x_in[i * P:(i + 1) * P, :])
                    nc.sync.dma_start(out_d[i * P:(i + 1) * P, :], t0_[:])

    nc.compile()
    return nc


def make_const_inputs():
    ident = np.eye(P, dtype=np.float32)
    # S^T diagonal block mask: valid iff tq_local >= tk_local
    mask = np.triu(np.ones((P, P), dtype=np.float32))
    # broadcast selector: denom row 64 -> all 64 output rows
    sel = np.zeros((P, P), dtype=np.float32)
    sel[64, 0:64] = 1.0
    return ident, mask, sel


_CACHE = {}


def _get_program(T, C, H, with_bias_attn, with_bias_proj, n_cores):
    key = (T, C, H, with_bias_attn, with_bias_proj, n_cores)
    if key not in _CACHE:
        _CACHE[key] = build_program(T=T, C=C, H=H, n_cores=n_cores,
                                    with_bias_attn=with_bias_attn,
                                    with_bias_proj=with_bias_proj)
    return _CACHE[key]


def kernel(x, w_attn, b_attn, w_proj, b_proj):
    x = np.ascontiguousarray(np.asarray(x, dtype=np.float32))
    w_attn = np.ascontiguousarray(np.asarray(w_attn, dtype=np.float32))
    w_proj = np.ascontiguousarray(np.asarray(w_proj, dtype=np.float32))
    b_attn = np.asarray(b_attn, dtype=np.float32)
    b_proj = np.asarray(b_proj, dtype=np.float32)
    B, T, C = x.shape
    H = 16
    n_cores = 8
    assert B == n_cores

    wba = bool(np.any(b_attn != 0))
    wbp = bool(np.any(b_proj != 0))
    nc = _get_program(T, C, H, wba, wbp, n_cores)

    ident, mask, sel = make_const_inputs()
    in_maps = []
    for i in range(n_cores):
        m = {"x": x[i], "w_attn": w_attn, "w_proj": w_proj,
             "ident": ident, "mask": mask, "sel": sel}
        if wba:
            m["b_attn"] = b_attn.reshape(1, -1)
        if wbp:
            m["b_proj"] = b_proj.reshape(1, -1)
        in_maps.append(m)

    res = run_bass_kernel_spmd(nc, in_maps, list(range(n_cores)))
    return np.stack([res.results[i]["out"] for i in range(n_cores)], axis=0)
